# revision 17
# baseline (speedup 1.0000x reference)
"""CliffordLinear (Cl(3,0)) Trainium2 kernel — fp8 DoubleRow edition.

Math: Cl(3,0) is isomorphic to the 2x2 complex matrices via the Pauli
representation phi.  The per-channel Clifford contraction maps to one
complex matrix product  phi(Out)[:,c] = phi(W) @ phi(X)[:,c]  per output
column c in {0,1}: for each c the real [b x 512] panels XRe, XIm against
the real 512x512 planes R, I of phi(W), via Gauss's 3-mult trick:

    G1 = XRe@R   G2 = XIm@I   G3 = (XRe+XIm)@(R+I)
    Re = G1 - G2                Im = G3 - G1 - G2

The PE runs every product as fp8e4 (e4m3) matmuls in DoubleRow perf
mode: one instruction carries TWO (stationary, moving) slot pairs and
costs 0.5 cycles per output row -- 2x the bf16 MAC rate.  bf16-grade
precision is kept by splitting every operand into an exact hi+lo pair
of e4m3 values (a = a_hi + a_lo with a_lo = fp8(a - fp8(a))) and
accumulating the three Karatsuba slot-products

    a_hi*b_hi + a_lo*b_hi + a_hi*b_lo        (a_lo*b_lo ~ 5e-4, dropped)

into the same PSUM bank.  Per G-product that is 12 slot-products = 6
DoubleRow instructions; 9216 PE cycles per 128-row b-tile (3.84 us) vs
12288 bf16 (5.12 us).  Measured end-to-end rel error 4.1e-3 (gate 2e-2).

HBM per core: x ships as three e4m3 hi/lo pair panels (XRe, XIm, XSum --
the Gauss sum panel is pre-split on host since fp8 splitting is
nonlinear) = 6.3 MB, planes R, I, S=R+I as hi/lo pairs = 1.5 MB, output
4.2 MB bf16; 12.0 MB ~ 33.3 us at 360 B/ns, co-saturated with the
30.7 us PE floor.

Per b-tile: G1 and G2 share one two-bank PSUM tile (sequential groups)
so ScalarE evicts both with ONE wide copy (1.04 us vs 2x612); Pool
copies G3 (711 ns); DVE does the Gauss recombine and four dual-blade
butterfly ops (bf16 2x mode, ~330 ns each).  A 24-matmul warmup chain
keeps the PE p-state ramp alive through the startup DMA wait.  Weight
DMAs ride the ACT HWDGE queue, x loads the SP queue, steady stores the
gpsimd SWDGE queue.  The last b-tile phases Re blades (need only G1/G2)
before the G3 matmuls and stores them early; only Im blades trail.

Sharding: data-parallel over batch (1024 rows/core); weights replicated.
"""

import sys

sys.path.insert(0, "/opt/trn_rl_repo")

import numpy as np

import concourse.bass as bass  # noqa: F401  (registers lowerings)
import concourse.mybir as mybir
import concourse.tile as tile
from concourse import bacc
from concourse.bass_utils import run_bass_kernel_spmd

N_CORES = 8
B, CIN, COUT, NB = 8192, 256, 256, 8
BS = B // N_CORES          # 1024 batch rows per core
BT = BS // 128             # 8 b-tiles
KP = 2 * CIN               # 512 contraction rows per panel (i,m)
PKT = KP // 128            # 4 k-tiles per panel
OUTW = COUT * NB           # 2048 output cols (blade-major: col = blade*256+o)
XCOLS = 3 * 2 * KP         # panel(re,im,sum) x lvl(hi,lo) x 4kt x 128b
WCOLS = PKT * 2 * 512      # kt x lvl(hi,lo) x 512 o-cols
SW = 6                     # w-plane scale 2^SW (fp8 range centering)

_cached = {}

N_WARM = 21                # PE p-state ramp chain during startup DMA wait


def _rw_ap(base, off, dims):
    """Clone `base` keeping its leading (partition) dim, replacing the free
    dims with `dims` [(step, num), ...] and adding `off` elements."""
    a = base.copy()
    part = a.ap.to_list()[0]
    v = a.ap
    v.clear()
    v.extend([tuple(part)] + [tuple(d) for d in dims])
    a.offset = a.offset + off
    return a


def _build_nc():
    bf = mybir.dt.bfloat16
    f8 = mybir.dt.float8e4
    f32 = mybir.dt.float32
    DR = mybir.MatmulPerfMode.DoubleRow
    nc = bacc.Bacc("TRN2", target_bir_lowering=False, debug=False,
                   num_devices=N_CORES)
    # x per phi-column c: [bt, p, col] with col = panel*1024 + lvl*512 +
    # kt*128 + b, panel in {XRe, XIm, XSum}, lvl in {hi, lo}, kappa = kt*128+p
    xt = [nc.dram_tensor(f"xt{c}", [BT, 128, XCOLS], f8,
                         kind="ExternalInput") for c in range(2)]
    # weight planes [R, I, S]: [p, col] with col = kt*1024 + lvl*512 + o
    wt = nc.dram_tensor("wt", [3, 128, WCOLS], f8, kind="ExternalInput")
    out = nc.dram_tensor("out", [BS, OUTW], bf, kind="ExternalOutput")

    with tile.TileContext(nc) as tc:
        with tc.tile_pool(name="wpool", bufs=1) as wpool, \
             tc.tile_pool(name="xpool", bufs=3) as xpool, \
             tc.tile_pool(name="epool", bufs=2) as epool, \
             tc.tile_pool(name="pspool", bufs=1, space="PSUM") as pspool:
            # PE warmup chain: long enough to span the startup DMA wait so
            # the p-state ramp (3 us continuous) is done when data lands.
            warm_in = wpool.tile([128, 384], bf, tag="warm_in")
            nc.vector.memset(warm_in[:].bitcast(mybir.dt.uint32), 0)
            warm_ps = pspool.tile([128, 512], f32, tag="gw")

            def _warm(n):
                for _ in range(n):
                    nc.tensor.matmul(warm_ps[:, 0:256], warm_in[:, :128],
                                     warm_in[:, 128:384], start=True,
                                     stop=True)

            _warm(N_WARM)

            # Startup DMA order = first-use order; w rides the ACT HWDGE
            # queue, x the SP queue, so DGE setup overheads overlap.
            w_t = [None] * 3
            x0_t = [None] * 2

            def _w_dma(p, queue):
                w_t[p] = wpool.tile([128, WCOLS], f8, tag=f"w{p}",
                                    name=f"w{p}")
                queue.dma_start(w_t[p][:], wt[p])

            def _x_dma(bt, c):
                t = xpool.tile([128, XCOLS], f8, tag=f"x{c}", name=f"x{c}")
                nc.sync.dma_start(t[:], xt[c][bt])
                return t

            # R, I ride the ACT queue; the DMA engine round-robins between
            # queues, so S must go on the SP queue AHEAD of the bt1
            # prefetches or they jump it (bt0's G3 would stall ~1 us).
            _w_dma(0, nc.scalar)
            x0_t[0] = _x_dma(0, 0)
            _w_dma(1, nc.scalar)
            x0_t[1] = _x_dma(0, 1)
            _w_dma(2, nc.sync)

            def emit_g(ps_ap, g, xtile, start):
                """One Gauss product G_g: 6 DoubleRow insts accumulating 12
                Karatsuba slot-products into psum region `ps_ap`."""
                combos = [(kp, xl, wl) for kp in range(2)
                          for (xl, wl) in ((0, 0), (1, 0), (0, 1))]
                for i, (kp, xl, wl) in enumerate(combos):
                    lhs = _rw_ap(xtile[:], g * 1024 + xl * 512 + kp * 256,
                                 [(128, 2), (1, 128)])
                    rhs = _rw_ap(w_t[g][:], kp * 2048 + wl * 512,
                                 [(1024, 2), (1, 512)])
                    nc.tensor.matmul(ps_ap, lhs, rhs,
                                     start=(start and i == 0),
                                     stop=(i == len(combos) - 1),
                                     perf_mode=DR)

            cur_x = x0_t
            for bt in range(BT):
                if bt + 1 < BT:
                    nxt_x = [_x_dma(bt + 1, 0), _x_dma(bt + 1, 1)]

                last = bt == BT - 1
                # G1+G2 share a two-bank psum tile (sequential groups) for
                # the single wide eviction; G3 gets its own bank.
                p12 = [pspool.tile([128, 1024], f32, tag=f"p12{c}",
                                   name=f"p12{c}") for c in range(2)]
                g3 = [pspool.tile([128, 512], f32, tag=f"g3{c}",
                                  name=f"g3{c}") for c in range(2)]

                def g12(c, g):
                    emit_g(p12[c][:, g * 512:(g + 1) * 512], g, cur_x[c],
                           True)

                # bt0 runs g-major to match DMA arrival with warm bursts
                # filling the DMA waits (keeps the PE p-state ramp alive);
                # the last bt runs c1 first so its Re path starts early;
                # steady bts run c-major.
                if bt == 0:
                    g12(0, 0)
                    g12(1, 0)
                    _warm(6)
                    g12(0, 1)
                    g12(1, 1)
                elif last:
                    g12(0, 0)
                    g12(0, 1)
                    g12(1, 0)
                    g12(1, 1)
                else:
                    for c in range(2):
                        g12(c, 0)
                        g12(c, 1)

                # Eviction part 1 (Re path; needs only G1/G2): one wide ACT
                # copy per c, then DVE recombine t = G1-G2, u = G1+G2.
                gs12 = [epool.tile([128, 1024], bf, tag=f"gs12{c}",
                                   name=f"gs12{c}") for c in range(2)]
                t_c = [epool.tile([128, 1024], bf, tag=f"t{c}",
                                  name=f"t{c}") for c in range(2)]
                u_c = [epool.tile([128, 512], bf, tag=f"u{c}",
                                  name=f"u{c}") for c in range(2)]
                for c in (0, 1):
                    nc.scalar.copy(gs12[c][:], p12[c][:])
                    nc.vector.tensor_sub(t_c[c][:, 0:512], gs12[c][:, 0:512],
                                         gs12[c][:, 512:1024])
                    nc.vector.tensor_add(u_c[c][:], gs12[c][:, 0:512],
                                         gs12[c][:, 512:1024])

                add, sub = nc.vector.tensor_add, nc.vector.tensor_sub
                inner = (1, 256)
                stage = epool.tile([128, OUTW], bf, tag="stage")
                orows = out[bt * 128:(bt + 1) * 128, 0:OUTW]

                if last:
                    # Re blades before the G3 matmuls:
                    #   x0 = ReA+ReD  x4 = ReA-ReD  (A,D = r0 of c0, r1 of c1)
                    #   x1 = ReC+ReB  x5 = ReC-ReB  (C,B = r1 of c0, r0 of c1)
                    nc.gpsimd.tensor_add(stage[:, 0:256], t_c[0][:, 0:256],
                                         t_c[1][:, 256:512])
                    nc.gpsimd.tensor_sub(stage[:, 1024:1280],
                                         t_c[0][:, 0:256],
                                         t_c[1][:, 256:512])
                    add(stage[:, 256:512], t_c[0][:, 256:512],
                        t_c[1][:, 0:256])
                    sub(stage[:, 1280:1536], t_c[0][:, 256:512],
                        t_c[1][:, 0:256])
                    nc.sync.dma_start(
                        _rw_ap(orows, 0, [(1024, 2), (1, 512)]),
                        _rw_ap(stage[:], 0, [(1024, 2), (1, 512)]))

                emit_g(g3[0][:], 2, cur_x[0], True)
                if last:
                    # G3c1 in two sequential half-groups so the Im tail
                    # pipelines at 256-col granularity.
                    for h in range(2):
                        combos = [(kp, xl, wl) for kp in range(2)
                                  for (xl, wl) in ((0, 0), (1, 0), (0, 1))]
                        for i, (kp, xl, wl) in enumerate(combos):
                            lhs = _rw_ap(cur_x[1][:],
                                         2 * 1024 + xl * 512 + kp * 256,
                                         [(128, 2), (1, 128)])
                            rhs = _rw_ap(w_t[2][:],
                                         kp * 2048 + wl * 512 + h * 256,
                                         [(1024, 2), (1, 256)])
                            nc.tensor.matmul(g3[1][:, h * 256:(h + 1) * 256],
                                             lhs, rhs, start=(i == 0),
                                             stop=(i == len(combos) - 1),
                                             perf_mode=DR)
                else:
                    emit_g(g3[1][:], 2, cur_x[1], True)

                # Im = G3 - (G1+G2): c0 fused on DVE (one PSUM-operand
                # read) on steady bts; the last bt routes G3 through ACT
                # copies so every DVE sub runs all-bf16 at 2x rate.
                if last:
                    gs30 = epool.tile([128, 512], bf, tag="gs30",
                                      name="gs30")
                    nc.scalar.copy(gs30[:], g3[0][:])
                    nc.vector.tensor_sub(t_c[0][:, 512:1024], gs30[:],
                                         u_c[0][:])
                else:
                    nc.vector.tensor_sub(t_c[0][:, 512:1024], g3[0][:],
                                         u_c[0][:])
                if not last:
                    gs31 = epool.tile([128, 512], bf, tag="gs31",
                                      name="gs31")
                    nc.scalar.copy(gs31[:], g3[1][:])
                    nc.vector.tensor_sub(t_c[1][:, 512:1024], gs31[:],
                                         u_c[1][:])
                    # Dual-blade butterfly: each op covers an (Re, Im)
                    # blade pair via 2-dim APs; one rides the Pool engine.
                    add(_rw_ap(stage[:], 0 * 256, [(1792, 2), inner]),
                        _rw_ap(t_c[0][:], 0, [(512, 2), inner]),
                        _rw_ap(t_c[1][:], 256, [(512, 2), inner]))
                    sub(_rw_ap(stage[:], 4 * 256, [(-256, 2), inner]),
                        _rw_ap(t_c[0][:], 0, [(512, 2), inner]),
                        _rw_ap(t_c[1][:], 256, [(512, 2), inner]))
                    nc.gpsimd.tensor_add(
                        _rw_ap(stage[:], 1 * 256, [(1280, 2), inner]),
                        _rw_ap(t_c[0][:], 256, [(512, 2), inner]),
                        _rw_ap(t_c[1][:], 0, [(512, 2), inner]))
                    sub(_rw_ap(stage[:], 5 * 256, [(-768, 2), inner]),
                        _rw_ap(t_c[0][:], 256, [(512, 2), inner]),
                        _rw_ap(t_c[1][:], 0, [(512, 2), inner]))
                    # Steady stores ride gpsimd's SWDGE queue: a store's
                    # sem wait on the stage tile must never head-of-line
                    # block the ACT copies or SP loads behind it.
                    nc.gpsimd.dma_start(orows, stage[:])
                else:
                    # Im tail in pipelined halves: ImB (r0 of c1) feeds
                    # x6 = ImC+ImB, x2 = ImC-ImB; ImD feeds x7 = ImA+ImD,
                    # x3 = ImA-ImD.  ACT copies each G3c1 half at its
                    # matmul stop; DVE runs the bf16 subs and singles with
                    # the D-half chain first; quarter stores stream out on
                    # the scalar and sync queues as soon as each pair lands.
                    gs3l = epool.tile([128, 512], bf, tag="gs3l",
                                      name="gs3l")
                    nc.scalar.copy(gs3l[:, 0:256], g3[1][:, 0:256])
                    nc.scalar.copy(gs3l[:, 256:512], g3[1][:, 256:512])
                    nc.vector.tensor_sub(t_c[1][:, 512:768],
                                         gs3l[:, 0:256],
                                         u_c[1][:, 0:256])
                    add(stage[:, 1536:1792], t_c[0][:, 768:1024],
                        t_c[1][:, 512:768])
                    sub(stage[:, 512:768], t_c[0][:, 768:1024],
                        t_c[1][:, 512:768])
                    nc.scalar.dma_start(
                        _rw_ap(orows, 512, [(1024, 2), (1, 256)]),
                        _rw_ap(stage[:], 512, [(1024, 2), (1, 256)]))
                    nc.vector.tensor_sub(t_c[1][:, 768:1024],
                                         gs3l[:, 256:512],
                                         u_c[1][:, 256:512])
                    add(stage[:, 1792:2048], t_c[0][:, 512:768],
                        t_c[1][:, 768:1024])
                    sub(stage[:, 768:1024], t_c[0][:, 512:768],
                        t_c[1][:, 768:1024])
                    nc.sync.dma_start(
                        _rw_ap(orows, 768, [(1024, 2), (1, 256)]),
                        _rw_ap(stage[:], 768, [(1024, 2), (1, 256)]))

                if bt + 1 < BT:
                    cur_x = nxt_x
    nc.finalize()
    return nc


def _pauli_parts(v):
    """v[..., 8] -> c0, c1 of shape [..., 2(m/r), 2(reim)]: the c-th column
    (Re, Im) of phi(v).  phi entries: A=P00=(v0+v4)+i(v3+v7),
    B=P01=(v1-v5)+i(v6-v2), C=P10=(v1+v5)+i(v6+v2), D=P11=(v0-v4)+i(v7-v3)."""
    c0 = np.empty(v.shape[:-1] + (2, 2), dtype=v.dtype)
    c1 = np.empty_like(c0)
    v0, v1, v2, v3, v4, v5, v6, v7 = (v[..., a] for a in range(8))
    c0[..., 0, 0] = v0 + v4   # Re A
    c0[..., 0, 1] = v3 + v7   # Im A
    c0[..., 1, 0] = v1 + v5   # Re C
    c0[..., 1, 1] = v6 + v2   # Im C
    c1[..., 0, 0] = v1 - v5   # Re B
    c1[..., 0, 1] = v6 - v2   # Im B
    c1[..., 1, 0] = v0 - v4   # Re D
    c1[..., 1, 1] = v7 - v3   # Im D
    return c0, c1


def _np_f8():
    return mybir.dt.np(mybir.dt.float8e4)


def _split8(a):
    """f32 -> (hi, lo) e4m3 pair with hi + lo ~ a (7-bit-mantissa grade)."""
    f8 = _np_f8()
    hi = a.astype(f8)
    lo = (a - hi.astype(np.float32)).astype(f8)
    return hi, lo


def _prep_w(weight):
    """weight [COUT, CIN, 8] -> [3, 128, WCOLS] e4m3 planes [R, I, S=R+I],
    each as hi/lo pairs, rows kappa=(i,m) folded to [kt, p], cols r-major
    (col = r*256 + o), 0.5 * 2^SW folded in."""
    w = weight.astype(np.float32)
    cw0, cw1 = _pauli_parts(w)    # cw_m[o, i, r, (re,im)] = phi(W[o,i])[r,m]
    R = np.empty((CIN, 2, 2, COUT), np.float32)   # [(i,m),(r,o)]
    I = np.empty_like(R)
    scale = 0.5 * 2.0 ** SW
    for m, cm in ((0, cw0), (1, cw1)):
        for r in range(2):
            R[:, m, r, :] = scale * cm[:, :, r, 0].T
            I[:, m, r, :] = scale * cm[:, :, r, 1].T
    planes = []
    Rm = R.reshape(KP, 512)
    Im_ = I.reshape(KP, 512)
    for P in (Rm, Im_, Rm + Im_):
        hi, lo = _split8(P)
        # [KP, 512] -> [kt, 128p, 512] -> [128p, kt, lvl, 512]
        arr = np.stack([hi.reshape(PKT, 128, 512), lo.reshape(PKT, 128, 512)],
                       axis=2)                      # [kt, 128p, lvl, 512]
        arr = arr.transpose(1, 0, 2, 3).reshape(128, WCOLS)
        planes.append(arr)
    return np.ascontiguousarray(np.stack(planes, axis=0))


def _prep_x(x):
    """x [B, CIN, 8] -> per-c arrays [N_CORES][BT, 128, XCOLS] e4m3: panels
    (XRe, XIm, XSum) x (hi, lo), device layout col = panel*1024 + lvl*512 +
    kt*128 + b with kappa = kt*128 + p."""
    xf = x.astype(np.float32)
    c0, c1 = _pauli_parts(xf)          # [B, CIN, m, reim]
    outs = []
    for arr in (c0, c1):
        re = arr[..., 0].reshape(B, KP)          # kappa = i*2+m
        im = arr[..., 1].reshape(B, KP)
        panels = np.stack([re, im, re + im], axis=1)   # [B, 3, KP]
        hi, lo = _split8(panels)
        a = np.stack([hi, lo], axis=2)           # [B, 3, lvl, KP]
        a = a.reshape(N_CORES, BT, 128, 3, 2, PKT, 128)  # [..b, pan, lvl, kt, p]
        a = a.transpose(0, 1, 6, 3, 4, 5, 2)     # [core, bt, p, pan, lvl, kt, b]
        outs.append(np.ascontiguousarray(
            a.reshape(N_CORES, BT, 128, XCOLS)))
    return outs


def kernel(x, weight, bias, cayley):
    assert x.shape == (B, CIN, NB) and weight.shape == (COUT, CIN, NB)
    if "nc" not in _cached:
        _cached["nc"] = _build_nc()
    nc = _cached["nc"]

    xp = _prep_x(np.asarray(x))
    wp = _prep_w(np.asarray(weight))
    in_maps = [{"xt0": xp[0][c], "xt1": xp[1][c], "wt": wp}
               for c in range(N_CORES)]
    res = run_bass_kernel_spmd(nc, in_maps, core_ids=list(range(N_CORES)))
    out = np.concatenate(
        [np.asarray(res.results[c]["out"]).astype(np.float32)
         for c in range(N_CORES)], axis=0)
    # cols are blade-major (blade*256 + o) -> [B, COUT, NB]; undo 2^SW
    out = out.reshape(B, NB, COUT).transpose(0, 2, 1) * 2.0 ** (-SW)
    out = out + np.asarray(bias, np.float32)[None]
    return np.ascontiguousarray(out.astype(np.float32))
